# revision 5
# baseline (speedup 1.0000x reference)
"""Trainium2 Bass kernel for nn_CombinedCriterionAEImpulse (retrieval_knn).

Computes, on 8 NeuronCores, the heavy part of the loss:
  - q[i, j]      = 2*p_i . g_j - |g_j|^2  over the full (8192, 32768) pred x gt
    grid (row max of q  <=>  row min of squared distance), reduced on-device to
    per-row maxes over groups of 64 columns.
  - qself[i, j]  = 2*p_i . p_j - |p_j|^2  over (8192, 8192) pred x pred with the
    diagonal masked, reduced the same way (groups of 64).
Rows (pred points) are sharded across the 8 cores; each core also emits the
group-level maxima.  The host then resolves the winning 64-wide group per row
(trivial numpy), gathers gt points/normals, and combines the scalar loss terms.
"""

import numpy as np

try:
    import concourse.bass as bass
except ImportError:  # pragma: no cover
    import sys

    sys.path.insert(0, "/opt/trn_rl_repo")
    import concourse.bass as bass

import concourse.mybir as mybir
import concourse.tile as tile
from concourse import bacc
from concourse.bass_utils import run_bass_kernel_spmd

P = 128
F32 = mybir.dt.float32

NPRED = 8192
NGT = 32768
NCORES = 8
RPC = NPRED // NCORES  # rows per core = 1024
BLOCKS = RPC // P  # 8
G = 64  # group size for on-device segmented max
ST = 2048  # supertile columns (4 PSUM banks)
CHUNK = 8192  # yt streaming chunk columns
DVE_EIGHTHS = 8  # of each 8 supertiles, this many reduce on DVE (rest ACT+POOL)

GL_GROUPS = NGT // G  # 512
GN_GROUPS = NPRED // G  # 128

ALPHA = 100.0
MARGIN = 0.3
EPS = 1e-05

# set by test harness to capture a profile
TRACE = False
LAST_RESULTS = None


def _build_kernel():
    nc = bacc.Bacc("TRN2", debug=False, enable_asserts=False)

    xt = nc.dram_tensor("xt", [4, RPC], F32, kind="ExternalInput").ap()
    yt = nc.dram_tensor("yt", [4, NGT], F32, kind="ExternalInput").ap()
    pt = nc.dram_tensor("pt", [4, NPRED], F32, kind="ExternalInput").ap()
    dmask = nc.dram_tensor("dmask", [P, P], F32, kind="ExternalInput").ap()
    gl = nc.dram_tensor("gl", [P, BLOCKS * GL_GROUPS], F32, kind="ExternalOutput").ap()
    gn = nc.dram_tensor("gn", [P, BLOCKS * GN_GROUPS], F32, kind="ExternalOutput").ap()

    n_chunks = NGT // CHUNK
    st_per_chunk = CHUNK // ST
    nxn_st = NPRED // ST
    st_groups = ST // G  # groups per supertile = 32

    with tile.TileContext(nc) as tc:
        with (
            tc.tile_pool(name="consts", bufs=1) as consts,
            tc.tile_pool(name="ytp", bufs=2) as ytp,
            tc.tile_pool(name="psum", bufs=2, space="PSUM") as psum,
            tc.tile_pool(name="copyb", bufs=3) as copyb,
            tc.tile_pool(name="treea", bufs=3) as treea,
            tc.tile_pool(name="treeb", bufs=3) as treeb,
            tc.tile_pool(name="acc", bufs=1) as accp,
        ):
            xt_s = consts.tile([4, RPC], F32, tag="xt")
            nc.sync.dma_start(xt_s[:], xt)
            pt_s = consts.tile([4, NPRED], F32, tag="pt")
            nc.sync.dma_start(pt_s[:], pt)
            dm_s = consts.tile([P, P], F32, tag="dm")
            nc.sync.dma_start(dm_s[:], dmask)

            glall = accp.tile([P, BLOCKS * GL_GROUPS], F32, tag="glall")
            gnall = accp.tile([P, BLOCKS * GN_GROUPS], F32, tag="gnall")
            nc.gpsimd.memset(glall[:], 0.0)
            nc.gpsimd.memset(gnall[:], 0.0)

            st_ctr = [0]

            def consume(ps, out_slice):
                """Segmented max: psum supertile [P, ST] -> out_slice [P, ST//G]."""
                use_dve = (st_ctr[0] % 8) < DVE_EIGHTHS
                st_ctr[0] += 1
                if use_dve:
                    nc.vector.tensor_reduce(
                        out=out_slice,
                        in_=ps.rearrange("p (g k) -> p g k", k=G),
                        axis=mybir.AxisListType.X,
                        op=mybir.AluOpType.max,
                    )
                    return
                cp = copyb.tile([P, ST], F32, tag="cp")
                nc.scalar.copy(out=cp[:], in_=ps)
                # grouped pairwise-max tree (ping-pong) down to width 1
                ta = treea.tile([P, ST // 2], F32, tag="ta")
                tb = treeb.tile([P, ST // 4], F32, tag="tb")
                w = G
                src = cp
                dsts = [ta, tb]
                d_i = 0
                while w > 1:
                    hw = w // 2
                    sv = src[:, : st_groups * w].rearrange("p (g w) -> p g w", w=w)
                    dst = dsts[d_i] if hw > 1 else None
                    if dst is not None:
                        dv = dst[:, : st_groups * hw].rearrange(
                            "p (g w) -> p g w", w=hw
                        )
                    else:
                        dv = out_slice.rearrange("p (g w) -> p g w", w=1)
                    nc.gpsimd.tensor_tensor(
                        out=dv, in0=sv[:, :, :hw], in1=sv[:, :, hw:],
                        op=mybir.AluOpType.max,
                    )
                    src = dst
                    d_i ^= 1
                    w = hw

            # ---- pred x gt ----
            for c in range(n_chunks):
                yt_s = ytp.tile([4, CHUNK], F32, tag="yt")
                nc.sync.dma_start(yt_s[:], yt[:, c * CHUNK : (c + 1) * CHUNK])
                for r in range(BLOCKS):
                    for s in range(st_per_chunk):
                        ps = psum.tile([P, ST], F32, tag="ps")
                        for m in range(ST // 512):
                            nc.tensor.matmul(
                                out=ps[:, m * 512 : (m + 1) * 512],
                                lhsT=xt_s[:, r * P : (r + 1) * P],
                                rhs=yt_s[:, s * ST + m * 512 : s * ST + (m + 1) * 512],
                                start=True,
                                stop=True,
                            )
                        base = r * GL_GROUPS + c * (CHUNK // G) + s * st_groups
                        consume(ps[:], glall[:, base : base + st_groups])

            # ---- pred x pred ---- (pt is rolled per-core: own rows at cols [0, RPC))
            for r in range(BLOCKS):
                for s in range(nxn_st):
                    ps = psum.tile([P, ST], F32, tag="ps")
                    for m in range(ST // 512):
                        nc.tensor.matmul(
                            out=ps[:, m * 512 : (m + 1) * 512],
                            lhsT=xt_s[:, r * P : (r + 1) * P],
                            rhs=pt_s[:, s * ST + m * 512 : s * ST + (m + 1) * 512],
                            start=True,
                            stop=True,
                        )
                    if s == (r * P) // ST:
                        off = (r * P) % ST
                        nc.vector.tensor_add(
                            out=ps[:, off : off + P],
                            in0=ps[:, off : off + P],
                            in1=dm_s[:],
                        )
                    base = r * GN_GROUPS + s * st_groups
                    consume(ps[:], gnall[:, base : base + st_groups])

            nc.sync.dma_start(out=gl, in_=glall[:])
            nc.sync.dma_start(out=gn, in_=gnall[:])
    nc.compile()
    return nc


_NC_CACHE = None


def _get_nc():
    global _NC_CACHE
    if _NC_CACHE is None:
        _NC_CACHE = _build_kernel()
    return _NC_CACHE


def kernel(pred_feat, pred_decoder, input_data, gt_data):
    global LAST_RESULTS
    pred_feat = np.asarray(pred_feat, dtype=np.float32)
    gt_data = np.asarray(gt_data, dtype=np.float32)
    pred = np.ascontiguousarray(pred_feat[:, :3])
    pred_n = np.ascontiguousarray(pred_feat[:, 3:])
    gt_pts = np.ascontiguousarray(gt_data[:, :3])
    gt_nrm = np.ascontiguousarray(gt_data[:, 3:])

    yt = np.concatenate([gt_pts.T, (gt_pts.astype(np.float64) ** 2).sum(1)[None]], 0)
    yt = np.ascontiguousarray(yt.astype(np.float32))
    dmask = np.zeros((P, P), np.float32)
    np.fill_diagonal(dmask, -1e30)

    in_maps = []
    for k in range(NCORES):
        prows = pred[k * RPC : (k + 1) * RPC]
        xt = np.concatenate([2.0 * prows.T, -np.ones((1, RPC), np.float32)], 0)
        rolled = np.roll(pred, -k * RPC, axis=0)
        pt = np.concatenate(
            [rolled.T, (rolled.astype(np.float64) ** 2).sum(1)[None]], 0
        )
        in_maps.append(
            {
                "xt": np.ascontiguousarray(xt.astype(np.float32)),
                "yt": yt,
                "pt": np.ascontiguousarray(pt.astype(np.float32)),
                "dmask": dmask,
            }
        )

    nc = _get_nc()
    res = run_bass_kernel_spmd(
        nc, in_maps, core_ids=list(range(NCORES)), trace=TRACE
    )
    LAST_RESULTS = res

    # ---- assemble per-row group maxima ----
    GL = np.empty((NPRED, GL_GROUPS), np.float32)
    GN = np.empty((NPRED, GN_GROUPS), np.float32)
    for k in range(NCORES):
        glk = res.results[k]["gl"].reshape(P, BLOCKS, GL_GROUPS)
        GL[k * RPC : (k + 1) * RPC] = glk.transpose(1, 0, 2).reshape(RPC, GL_GROUPS)
        gnk = res.results[k]["gn"].reshape(P, BLOCKS, GN_GROUPS)
        GN[k * RPC : (k + 1) * RPC] = gnk.transpose(1, 0, 2).reshape(RPC, GN_GROUPS)

    rows = np.arange(NPRED)

    # ---- nearest gt point: resolve winning group of 64 on host ----
    gstar = np.argmax(GL, axis=1)
    cand = gstar[:, None] * G + np.arange(G)[None, :]  # (NPRED, G)
    diff = pred[:, None, :] - gt_pts[cand]  # (NPRED, G, 3)
    d2 = np.einsum("ijk,ijk->ij", diff, diff)
    loc = np.argmin(d2, axis=1)
    jstar = cand[rows, loc]

    closest = gt_pts[jstar]
    attraction = np.mean(((pred - closest) ** 2).astype(np.float64))

    # ---- normal alignment ----
    cn = gt_nrm[jstar]
    pn_norm = np.maximum(np.sqrt((pred_n**2).sum(1, keepdims=True)), EPS)
    cn_norm = np.maximum(np.sqrt((cn**2).sum(1, keepdims=True)), EPS)
    cos = ((pred_n / pn_norm) * (cn / cn_norm)).sum(1)
    norm_loss = np.mean((1.0 - cos).astype(np.float64))

    # ---- repulsion: min distance to other pred points ----
    x2 = (pred.astype(np.float64) ** 2).sum(1)
    local = rows % RPC
    gc = local // G  # contaminated (diagonal-containing) group, in rolled coords
    core = rows // RPC
    GN2 = GN.copy()
    GN2[rows, gc] = -np.inf
    m1 = x2 - GN2.max(axis=1)  # min d^2 over all non-contaminated groups
    # recompute the contaminated group exactly (excluding self)
    candn = (gc[:, None] * G + np.arange(G)[None, :] + core[:, None] * RPC) % NPRED
    diffn = pred[:, None, :] - pred[candn]
    d2n = np.einsum("ijk,ijk->ij", diffn, diffn)
    d2n[candn == rows[:, None]] = np.inf
    m2 = d2n.min(axis=1)
    min_d2 = np.minimum(m1, m2)
    min_dist = np.sqrt(np.maximum(min_d2, 0.0))
    pen = np.logaddexp(0.0, ALPHA * (MARGIN - min_dist))
    repulsion = np.mean(pen**2)

    loss = attraction + repulsion + 10.0 * norm_loss
    return np.float32(loss)


# revision 6
# speedup vs baseline: 2.9707x; 2.9707x over previous
"""Trainium2 Bass kernel for nn_CombinedCriterionAEImpulse (retrieval_knn).

Computes, on 8 NeuronCores, the heavy part of the loss:
  - q[i, j]      = 2*p_i . g_j - |g_j|^2  over the full (8192, 32768) pred x gt
    grid (row max of q  <=>  row min of squared distance), reduced on-device to
    per-row maxes over groups of 64 columns.
  - qself[i, j]  = 2*p_i . p_j - |p_j|^2  over (8192, 8192) pred x pred with the
    diagonal masked, reduced the same way (groups of 64).
Rows (pred points) are sharded across the 8 cores; each core also emits the
group-level maxima.  The host then resolves the winning 64-wide group per row
(trivial numpy), gathers gt points/normals, and combines the scalar loss terms.
"""

import numpy as np

try:
    import concourse.bass as bass
except ImportError:  # pragma: no cover
    import sys

    sys.path.insert(0, "/opt/trn_rl_repo")
    import concourse.bass as bass

import concourse.mybir as mybir
import concourse.tile as tile
from concourse import bacc
from concourse.bass_utils import run_bass_kernel_spmd

P = 128
F32 = mybir.dt.float32
BF16 = mybir.dt.bfloat16
K = 11

NPRED = 8192
NGT = 32768
NCORES = 8
RPC = NPRED // NCORES  # rows per core = 1024
BLOCKS = RPC // P  # 8
G = 64  # group size for on-device segmented max
ST = 2048  # supertile columns (4 PSUM banks)
CHUNK = 8192  # yt streaming chunk columns
DVE_EIGHTHS = 8  # of each 8 supertiles, this many reduce on DVE (rest ACT+POOL)

GL_GROUPS = NGT // G  # 512
GN_GROUPS = NPRED // G  # 128

ALPHA = 100.0
MARGIN = 0.3
EPS = 1e-05

# set by test harness to capture a profile
TRACE = False
LAST_RESULTS = None


def _build_kernel():
    nc = bacc.Bacc("TRN2", debug=False, enable_asserts=False)

    xt = nc.dram_tensor("xt", [K, RPC], BF16, kind="ExternalInput").ap()
    yt = nc.dram_tensor("yt", [K, NGT], BF16, kind="ExternalInput").ap()
    pt = nc.dram_tensor("pt", [K, NPRED], BF16, kind="ExternalInput").ap()
    dmask = nc.dram_tensor("dmask", [P, P], F32, kind="ExternalInput").ap()
    gl = nc.dram_tensor("gl", [P, BLOCKS * GL_GROUPS], F32, kind="ExternalOutput").ap()
    gn = nc.dram_tensor("gn", [P, BLOCKS * GN_GROUPS], F32, kind="ExternalOutput").ap()

    n_chunks = NGT // CHUNK
    st_per_chunk = CHUNK // ST
    nxn_st = NPRED // ST
    st_groups = ST // G  # groups per supertile = 32

    with tile.TileContext(nc) as tc:
        with (
            tc.tile_pool(name="consts", bufs=1) as consts,
            tc.tile_pool(name="ytp", bufs=2) as ytp,
            tc.tile_pool(name="psum", bufs=2, space="PSUM") as psum,
            tc.tile_pool(name="copyb", bufs=3) as copyb,
            tc.tile_pool(name="treea", bufs=3) as treea,
            tc.tile_pool(name="treeb", bufs=3) as treeb,
            tc.tile_pool(name="acc", bufs=1) as accp,
        ):
            xt_s = consts.tile([K, RPC], BF16, tag="xt")
            nc.sync.dma_start(xt_s[:], xt)
            pt_s = consts.tile([K, NPRED], BF16, tag="pt")
            nc.sync.dma_start(pt_s[:], pt)
            dm_s = consts.tile([P, P], F32, tag="dm")
            nc.sync.dma_start(dm_s[:], dmask)

            glall = accp.tile([P, BLOCKS * GL_GROUPS], F32, tag="glall")
            gnall = accp.tile([P, BLOCKS * GN_GROUPS], F32, tag="gnall")
            nc.gpsimd.memset(glall[:], 0.0)
            nc.gpsimd.memset(gnall[:], 0.0)

            st_ctr = [0]

            def consume(ps, out_slice):
                """Segmented max: psum supertile [P, ST] -> out_slice [P, ST//G]."""
                use_dve = (st_ctr[0] % 8) < DVE_EIGHTHS
                st_ctr[0] += 1
                if use_dve:
                    nc.vector.tensor_reduce(
                        out=out_slice,
                        in_=ps.rearrange("p (g k) -> p g k", k=G),
                        axis=mybir.AxisListType.X,
                        op=mybir.AluOpType.max,
                    )
                    return
                cp = copyb.tile([P, ST], F32, tag="cp")
                nc.scalar.copy(out=cp[:], in_=ps)
                # grouped pairwise-max tree (ping-pong) down to width 1
                ta = treea.tile([P, ST // 2], F32, tag="ta")
                tb = treeb.tile([P, ST // 4], F32, tag="tb")
                w = G
                src = cp
                dsts = [ta, tb]
                d_i = 0
                while w > 1:
                    hw = w // 2
                    sv = src[:, : st_groups * w].rearrange("p (g w) -> p g w", w=w)
                    dst = dsts[d_i] if hw > 1 else None
                    if dst is not None:
                        dv = dst[:, : st_groups * hw].rearrange(
                            "p (g w) -> p g w", w=hw
                        )
                    else:
                        dv = out_slice.rearrange("p (g w) -> p g w", w=1)
                    nc.gpsimd.tensor_tensor(
                        out=dv, in0=sv[:, :, :hw], in1=sv[:, :, hw:],
                        op=mybir.AluOpType.max,
                    )
                    src = dst
                    d_i ^= 1
                    w = hw

            # ---- pred x gt ----
            for c in range(n_chunks):
                yt_s = ytp.tile([K, CHUNK], BF16, tag="yt")
                nc.sync.dma_start(yt_s[:], yt[:, c * CHUNK : (c + 1) * CHUNK])
                for r in range(BLOCKS):
                    for s in range(st_per_chunk):
                        ps = psum.tile([P, ST], F32, tag="ps")
                        for m in range(ST // 512):
                            nc.tensor.matmul(
                                out=ps[:, m * 512 : (m + 1) * 512],
                                lhsT=xt_s[:, r * P : (r + 1) * P],
                                rhs=yt_s[:, s * ST + m * 512 : s * ST + (m + 1) * 512],
                                start=True,
                                stop=True,
                            )
                        base = r * GL_GROUPS + c * (CHUNK // G) + s * st_groups
                        consume(ps[:], glall[:, base : base + st_groups])

            # ---- pred x pred ---- (pt is rolled per-core: own rows at cols [0, RPC))
            for r in range(BLOCKS):
                for s in range(nxn_st):
                    ps = psum.tile([P, ST], F32, tag="ps")
                    for m in range(ST // 512):
                        nc.tensor.matmul(
                            out=ps[:, m * 512 : (m + 1) * 512],
                            lhsT=xt_s[:, r * P : (r + 1) * P],
                            rhs=pt_s[:, s * ST + m * 512 : s * ST + (m + 1) * 512],
                            start=True,
                            stop=True,
                        )
                    if s == (r * P) // ST:
                        off = (r * P) % ST
                        nc.vector.tensor_add(
                            out=ps[:, off : off + P],
                            in0=ps[:, off : off + P],
                            in1=dm_s[:],
                        )
                    base = r * GN_GROUPS + s * st_groups
                    consume(ps[:], gnall[:, base : base + st_groups])

            nc.sync.dma_start(out=gl, in_=glall[:])
            nc.sync.dma_start(out=gn, in_=gnall[:])
    nc.compile()
    return nc


_NC_CACHE = None


def _get_nc():
    global _NC_CACHE
    if _NC_CACHE is None:
        _NC_CACHE = _build_kernel()
    return _NC_CACHE


def kernel(pred_feat, pred_decoder, input_data, gt_data):
    global LAST_RESULTS
    pred_feat = np.asarray(pred_feat, dtype=np.float32)
    gt_data = np.asarray(gt_data, dtype=np.float32)
    pred = np.ascontiguousarray(pred_feat[:, :3])
    pred_n = np.ascontiguousarray(pred_feat[:, 3:])
    gt_pts = np.ascontiguousarray(gt_data[:, :3])
    gt_nrm = np.ascontiguousarray(gt_data[:, 3:])

    import ml_dtypes

    bf = ml_dtypes.bfloat16

    def split_hi_lo(x):
        hi = x.astype(bf).astype(np.float32)
        lo = (x - hi).astype(bf).astype(np.float32)
        return hi, lo

    def rhs_rows(pts):
        """[K, n] moving-operand rows for target points pts (n, 3)."""
        hi, lo = split_hi_lo(pts)
        s = (pts.astype(np.float64) ** 2).sum(1).astype(np.float32)
        shi, slo = split_hi_lo(s)
        out = np.concatenate([hi.T, lo.T, hi.T, shi[None], slo[None]], 0)
        return np.ascontiguousarray(out.astype(bf))

    def lhs_rows(pts):
        """[K, n] stationary rows for query points pts (n, 3)."""
        hi, lo = split_hi_lo(pts)
        ones = np.ones((1, pts.shape[0]), np.float32)
        out = np.concatenate([2 * hi.T, 2 * hi.T, 2 * lo.T, -ones, -ones], 0)
        return np.ascontiguousarray(out.astype(bf))

    yt = rhs_rows(gt_pts)
    dmask = np.zeros((P, P), np.float32)
    np.fill_diagonal(dmask, -1e30)

    in_maps = []
    for k in range(NCORES):
        rolled = np.roll(pred, -k * RPC, axis=0)
        in_maps.append(
            {
                "xt": lhs_rows(pred[k * RPC : (k + 1) * RPC]),
                "yt": yt,
                "pt": rhs_rows(rolled),
                "dmask": dmask,
            }
        )

    nc = _get_nc()
    res = run_bass_kernel_spmd(
        nc, in_maps, core_ids=list(range(NCORES)), trace=TRACE
    )
    LAST_RESULTS = res

    # ---- assemble per-row group maxima ----
    GL = np.empty((NPRED, GL_GROUPS), np.float32)
    GN = np.empty((NPRED, GN_GROUPS), np.float32)
    for k in range(NCORES):
        glk = res.results[k]["gl"].reshape(P, BLOCKS, GL_GROUPS)
        GL[k * RPC : (k + 1) * RPC] = glk.transpose(1, 0, 2).reshape(RPC, GL_GROUPS)
        gnk = res.results[k]["gn"].reshape(P, BLOCKS, GN_GROUPS)
        GN[k * RPC : (k + 1) * RPC] = gnk.transpose(1, 0, 2).reshape(RPC, GN_GROUPS)

    rows = np.arange(NPRED)

    # ---- nearest gt point: resolve winning group of 64 on host ----
    gstar = np.argmax(GL, axis=1)
    cand = gstar[:, None] * G + np.arange(G)[None, :]  # (NPRED, G)
    diff = pred[:, None, :] - gt_pts[cand]  # (NPRED, G, 3)
    d2 = np.einsum("ijk,ijk->ij", diff, diff)
    loc = np.argmin(d2, axis=1)
    jstar = cand[rows, loc]

    closest = gt_pts[jstar]
    attraction = np.mean(((pred - closest) ** 2).astype(np.float64))

    # ---- normal alignment ----
    cn = gt_nrm[jstar]
    pn_norm = np.maximum(np.sqrt((pred_n**2).sum(1, keepdims=True)), EPS)
    cn_norm = np.maximum(np.sqrt((cn**2).sum(1, keepdims=True)), EPS)
    cos = ((pred_n / pn_norm) * (cn / cn_norm)).sum(1)
    norm_loss = np.mean((1.0 - cos).astype(np.float64))

    # ---- repulsion: min distance to other pred points ----
    x2 = (pred.astype(np.float64) ** 2).sum(1)
    local = rows % RPC
    gc = local // G  # contaminated (diagonal-containing) group, in rolled coords
    core = rows // RPC
    GN2 = GN.copy()
    GN2[rows, gc] = -np.inf
    m1 = x2 - GN2.max(axis=1)  # min d^2 over all non-contaminated groups
    # recompute the contaminated group exactly (excluding self)
    candn = (gc[:, None] * G + np.arange(G)[None, :] + core[:, None] * RPC) % NPRED
    diffn = pred[:, None, :] - pred[candn]
    d2n = np.einsum("ijk,ijk->ij", diffn, diffn)
    d2n[candn == rows[:, None]] = np.inf
    m2 = d2n.min(axis=1)
    min_d2 = np.minimum(m1, m2)
    min_dist = np.sqrt(np.maximum(min_d2, 0.0))
    pen = np.logaddexp(0.0, ALPHA * (MARGIN - min_dist))
    repulsion = np.mean(pen**2)

    loss = attraction + repulsion + 10.0 * norm_loss
    return np.float32(loss)
